# revision 1
# baseline (speedup 1.0000x reference)
"""Trainium2 Bass kernel for K[c,h,l] = sum_n W[c,h,n] * cos(Lambda_im[n] * l).

Shapes (hardcoded): W (1, 512, 4096) f32, Lambda_im (4096,) f32, L = 8192.
Output: (1, 512, 8192) f32.

Strategy: shard L across the 8 cores (1024 columns each). Each core
generates its slice of the cosine-Vandermonde matrix S[n, l] on-chip and
contracts it with W on the PE array (fp16 matmuls, fp32 PSUM
accumulation over the 4096-long n axis). Work per core is PE-bound at
~55us (256 matmuls of 128x128x512 @ ~216 ns warm).

Angle pipeline, per core c, chunk q (n = 128q..128q+127), j = 0..1023:
    f_n    = Lambda_im[n] / (2*pi)                       (host, f32)
    base_n = mod(Lambda_im[n]*(1024c)/(2*pi) + .25, 1)   (host f64 -> f32)
    r      = y - round(y),  y = f_n*j + base_n           (one fused custom
             DVE op, FRAC_AFFINE_ANT: the 2^23 add/sub RNE round trick,
             5 ALU slices, |r| <= 0.5 exactly)
    S[n,j] = sin(r * SIN_SCALE) = sin(2pi*(f_n*l + 1/4)) = cos(Lambda_im[n]*l)
             (ACT Sin, fused over 2 chunks, fp16 output; SIN_SCALE is one
             ulp under 2*pi so the argument stays inside Sin's [-pi, pi])
    out   += Wt_chunk.T @ S_chunk                        (PE, 8 psum banks)

The three stages (DMA+FRAC | Sin | matmuls) are software-pipelined by
hand across superchunks so the strict-FIFO engine queues never
head-of-line block on a cross-engine dependency. Weight DMAs are batched
(one descriptor per 2 chunks) and alternate between two DMA queues.
"""

import os

os.environ.setdefault("MYCRO_LOCAL_CACHE", "1")
# no NTFF hook in this container; never let a stray BASS_TRACE break the run
os.environ.setdefault("BASS_NEVER_TRACE", "1")

from contextlib import ExitStack

import numpy as np

import concourse.tile as tile
from concourse import bacc, mybir
from concourse.bass_utils import run_bass_kernel_spmd

N_CORES = 8
H = 512
N = 4096
L_FULL = 8192
P = 128
F = L_FULL // N_CORES  # 1024 columns of L per core
NCH = N // P  # 32 contraction chunks
SUP = 2  # chunks fused per ACT/STT pass
NSUP = NCH // SUP
HT = H // P  # 4 output row tiles
NHALF = 2  # two 512-wide moving halves per 1024 columns

F32 = mybir.dt.float32
F16 = mybir.dt.float16

MAGIC = float(2**23)
# sin argument window: scale one ulp under 2*pi so |r|<=0.5 maps inside
# the ScalarE Sin domain [-pi, pi].
SIN_SCALE = float(np.nextafter(np.float32(2 * np.pi), np.float32(0)))

_compiled = {}

_FRAC_OP = None


def _frac_affine_op():
    """One fused DVE op: out = y - round(y), y = in0*s0[p] + s1[p].

    round() is the fp32 RNE 2^23 magic (each DVE ALU slice rounds its
    fp32 result, so (y + 2^23) - 2^23 == round-to-nearest-even(y) for
    |y| < 2^22). Registered at runtime via the documented dve_ops
    extension point (append to OPS; row assigned past the last entry).
    """
    global _FRAC_OP
    if _FRAC_OP is not None:
        return _FRAC_OP
    from concourse import dve_ops
    from concourse.dve_spec import Spec, Src0, C0, C1, C2, lower, _has_src1
    from concourse.dve_uop import DveOpSpec

    name = "FRAC_AFFINE_ANT"
    for existing in dve_ops.OPS:
        if existing.name == name:
            _FRAC_OP = existing
            return existing

    y = Src0 * C0 + C1
    body = y - ((y + C2) - C2)

    def reference(in0, in1, s0, s1, imm2):
        m = np.float32(imm2)
        yv = (
            in0.astype(np.float32) * np.float32(s0) + np.float32(s1)
            if np.isscalar(s0) or np.ndim(s0) == 0
            else in0.astype(np.float32) * s0.astype(np.float32)
            + s1.astype(np.float32)
        )
        yv = yv.astype(np.float32)
        k = ((yv + m).astype(np.float32) - m).astype(np.float32)
        return (yv - k).astype(np.float32)

    spec = Spec(body=body, reference=reference)
    row = dve_ops._CUSTOM_DVE_ROW_BASE + len(dve_ops.OPS)
    assert row < 0x20, "custom-DVE row overflow"
    dve_ops._SUB_OPCODE_FOR_NAME[name] = row
    shas = {}
    for ver in ("v3", "v4"):
        s = DveOpSpec(
            name=name,
            opcode=row,
            uops=lower(spec, ver=ver),
            rd1_en=_has_src1(spec),
        )
        shas[ver] = s.sha(ver)
    op = dve_ops.DveOp(name, spec, subdim=False, uops_sha=shas)
    dve_ops.OPS.append(op)
    dve_ops.CUSTOM_DVE_SPECS[name] = spec
    _FRAC_OP = op
    return op


def _build(reps=1, mode="full"):
    nc = bacc.Bacc(
        "TRN2",
        target_bir_lowering=False,
        debug=False,
        num_devices=N_CORES,
    )
    wt = nc.dram_tensor("wt", [N, H], F16, kind="ExternalInput")
    fcol = nc.dram_tensor("fcol", [P, NCH], F32, kind="ExternalInput")
    basecol = nc.dram_tensor("basecol", [P, NCH], F32, kind="ExternalInput")
    iota = nc.dram_tensor("iota", [P, F], F32, kind="ExternalInput")
    out = nc.dram_tensor("out", [H, F], F32, kind="ExternalOutput")

    with tile.TileContext(nc) as tc:
        with ExitStack() as ctx:
            _body(
                ctx,
                tc,
                wt.ap(),
                fcol.ap(),
                basecol.ap(),
                iota.ap(),
                out.ap(),
                reps,
                mode,
            )
    nc.compile()
    return nc


def _body(ctx, tc, wt_ap, f_ap, base_ap, iota_ap, out_ap, reps, mode="full"):
    nc = tc.nc
    const = ctx.enter_context(tc.tile_pool(name="const", bufs=1))
    wtp = ctx.enter_context(tc.tile_pool(name="wt", bufs=8))
    sp = ctx.enter_context(tc.tile_pool(name="sgen", bufs=4))
    psp = ctx.enter_context(tc.tile_pool(name="ps", bufs=1, space="PSUM"))
    op = ctx.enter_context(tc.tile_pool(name="outp", bufs=4))

    do_dma = mode in ("full", "mm_only", "dma_only")
    do_sgen = mode in ("full", "sgen_only")
    do_mm = mode in ("full", "mm_only")
    if mode == "noop":
        do_dma = do_sgen = do_mm = False

    iota_sb = const.tile([P, F], F32, tag="iota")
    nc.sync.dma_start(iota_sb[:], iota_ap)
    f_sb = const.tile([P, NCH], F32, tag="f")
    nc.sync.dma_start(f_sb[:], f_ap)
    b_sb = const.tile([P, NCH], F32, tag="b")
    nc.sync.dma_start(b_sb[:], base_ap)
    magic_col = const.tile([P, 1], F32, tag="magic_col")
    nc.vector.memset(magic_col[:], MAGIC)
    # touch Sin once so the ACT table set loads during the DMA/pipeline fill
    sin_warm = const.tile([P, 1], F32, tag="sin_warm")
    nc.scalar.activation(
        sin_warm[:],
        magic_col[:],
        mybir.ActivationFunctionType.Sin,
        scale=0.0,
    )

    fixed_s = None
    if not do_sgen:
        fixed_s = const.tile([P, F * SUP], F16, tag="fixed_s")
        nc.vector.memset(fixed_s[:], 0.25)

    ps = {}
    if do_mm:
        for h in range(HT):
            for half in range(NHALF):
                ps[(h, half)] = psp.tile(
                    [P, 512], F32, tag=f"ps{h}_{half}", name=f"ps{h}_{half}"
                )

    W = F * SUP  # free width of one fused superchunk

    def body(rep):
        if mode == "noop":
            nc.vector.memset(magic_col[:], MAGIC)
            return
        # stage functions of the software pipeline, indexed by superchunk
        wts = {}
        negr = {}
        s_t = {}

        frac_op = _frac_affine_op()

        def st_load(sc):
            if do_sgen:
                negr[sc] = sp.tile([P, W], F32, tag="negr", name=f"nr_{sc}")
            if do_dma:
                # one batched DMA per superchunk: SUP chunk-rows of wt,
                # [P, SUP, H] view of DRAM -> [P, SUP*H] SBUF tile
                wt_t = wtp.tile([P, SUP * H], F16, tag="wt", name=f"wt_{sc}")
                src = wt_ap[sc * SUP * P : (sc + 1) * SUP * P, :].rearrange(
                    "(s p) h -> p s h", p=P
                )
                eng = (nc.sync, nc.gpsimd)[sc % 2]
                eng.dma_start(wt_t[:], src)
                wts[sc] = wt_t
            if do_sgen:
                for i in range(SUP):
                    q = sc * SUP + i
                    nc.vector._custom_dve(
                        frac_op,
                        out=negr[sc][:, i * F : (i + 1) * F],
                        in0=iota_sb[:],
                        s0=f_sb[:, q : q + 1],
                        s1=b_sb[:, q : q + 1],
                        imm2=MAGIC,
                    )

        def st_sin(sc):
            if not do_sgen:
                s_t[sc] = fixed_s
                return
            s_t[sc] = sp.tile([P, W], F16, tag="s", name=f"s_{sc}")
            nc.scalar.activation(
                s_t[sc][:],
                negr[sc][:],
                mybir.ActivationFunctionType.Sin,
                scale=SIN_SCALE,
            )
            del negr[sc]

        def st_mm(sc):
            if not do_mm:
                s_t.pop(sc, None)
                return
            for i in range(SUP):
                q = sc * SUP + i
                for h in range(HT):
                    lhsT = wts[sc][:, i * H + h * P : i * H + (h + 1) * P]
                    for half in range(NHALF):
                        nc.tensor.matmul(
                            ps[(h, half)][:],
                            lhsT,
                            s_t[sc][
                                :,
                                i * F + half * 512 : i * F + (half + 1) * 512,
                            ],
                            start=(q == 0),
                            stop=(q == NCH - 1),
                        )
            del wts[sc]
            s_t.pop(sc, None)

        stages = [st_load, st_sin, st_mm]
        depth = len(stages)
        for t in range(NSUP + depth - 1):
            for si in range(depth - 1, -1, -1):
                sc = t - si
                if 0 <= sc < NSUP:
                    stages[si](sc)

    if reps == 1:
        body(0)
    else:
        with tc.For_i(0, reps, 1):
            body(0)

    for h in range(HT):
        if not do_mm:
            break
        for half in range(NHALF):
            o = op.tile([P, 512], F32, tag="o", name=f"o_{h}_{half}")
            if h % 2 == 0:
                nc.scalar.copy(o[:], ps[(h, half)][:])
            else:
                nc.vector.tensor_copy(o[:], ps[(h, half)][:])
            (nc.sync, nc.gpsimd)[(h * NHALF + half) % 2].dma_start(
                out_ap[h * P : (h + 1) * P, half * 512 : (half + 1) * 512],
                o[:],
            )


def _prepare_inputs(W, Lambda_im):
    lam64 = np.asarray(Lambda_im, dtype=np.float64)
    f32 = (lam64 / (2 * np.pi)).astype(np.float32)
    fcol = np.ascontiguousarray(f32.reshape(NCH, P).T)
    wt = np.ascontiguousarray(
        np.asarray(W, dtype=np.float32)[0].T.astype(np.float16)
    )
    iota = np.ascontiguousarray(
        np.broadcast_to(np.arange(F, dtype=np.float32), (P, F))
    )
    in_maps = []
    for c in range(N_CORES):
        base64 = np.mod(lam64 * (F * c) / (2 * np.pi) + 0.25, 1.0)
        basecol = np.ascontiguousarray(
            base64.astype(np.float32).reshape(NCH, P).T
        )
        in_maps.append(
            {"wt": wt, "fcol": fcol, "basecol": basecol, "iota": iota}
        )
    return in_maps


def _run(W, Lambda_im, L, trace=False, reps=1, mode="full", **rbk_kwargs):
    assert int(L) == L_FULL, f"kernel hardcoded for L={L_FULL}, got {L}"
    key = (reps, mode)
    if key not in _compiled:
        _compiled[key] = _build(reps, mode)
    nc = _compiled[key]
    in_maps = _prepare_inputs(W, Lambda_im)
    res = run_bass_kernel_spmd(
        nc, in_maps, list(range(N_CORES)), trace=trace, **rbk_kwargs
    )
    K = np.empty((1, H, L_FULL), dtype=np.float32)
    for c in range(N_CORES):
        K[0, :, c * F : (c + 1) * F] = res.results[c]["out"]
    return K, res


def kernel(W, Lambda_im, L):
    K, _ = _run(W, Lambda_im, L)
    return K

